# revision 4
# baseline (speedup 1.0000x reference)
"""AttentionClustering kernel for Trainium2, 8 NeuronCores, data-parallel over batch.

Pipeline per core (one image, NCHW f32 in / f32 out):
  conv3x3(replicate pad) + relu  -> conv3x3(replicate pad) + relu -> 1x1 conv
  -> squared-distance logits vs 32 cluster centers -> softmax over clusters
  -> linear recombination with cluster_label.

Implementation notes:
  * Convs run as shifted matmuls accumulating in PSUM, fp16 inputs / f32 accum.
    q1 is stored twice in SBUF partitions (rows 64-127 shifted one image row)
    so the dy=0/dy=1 taps fuse into single K=128 matmuls.
  * softmax max-subtraction is algebraically unnecessary here: logits
    -(|q|^2 - 2 q.mu + |mu|^2) reduce (shift-invariance in the softmax) to
    2 q.mu - |mu|^2 which is always < 0 for this model family; exp() is safe
    in f32. |mu|^2 folds into the exp's per-partition bias.
  * Normalization 1/sum runs on 4-partition tiles; the per-pixel reciprocal is
    broadcast back over partitions with a DRAM round-trip DMA (step-0 AP).
"""
import sys

sys.path.insert(0, "/opt/trn_rl_repo")

import numpy as np
import ml_dtypes

import concourse.bass as bass
import concourse.mybir as mybir
from concourse import bacc, bass_utils
from concourse.tile import TileContext

F32 = mybir.dt.float32
F16 = mybir.dt.float16
BF16 = mybir.dt.bfloat16

B, CIN, H, W = 8, 3, 256, 256
Q, NC, COUT = 64, 32, 64
R = 16          # output rows per strip
S = H // R      # strips
ACT_F = mybir.ActivationFunctionType
ALU = mybir.AluOpType

_cache = {}


def _build():
    nc = bacc.Bacc()
    xpad_t = nc.dram_tensor("xpad", (CIN, H + 2, W + 2), F16, kind="ExternalInput")
    w1c_t = nc.dram_tensor("w1c", (27, Q), F16, kind="ExternalInput")
    wa_t = nc.dram_tensor("wa", (128, 384), F16, kind="ExternalInput")
    w3m_t = nc.dram_tensor("w3m", (Q, Q), F16, kind="ExternalInput")
    mu2_t = nc.dram_tensor("mu2", (Q, NC), F16, kind="ExternalInput")
    lb4_t = nc.dram_tensor("lb4", (128, COUT), BF16, kind="ExternalInput")
    ones_t = nc.dram_tensor("onesb", (128, 4), BF16, kind="ExternalInput")
    b1_t = nc.dram_tensor("b1c", (Q, 1), F32, kind="ExternalInput")
    b2_t = nc.dram_tensor("b2c", (Q, 1), F32, kind="ExternalInput")
    b3_t = nc.dram_tensor("b3c", (Q, 1), F32, kind="ExternalInput")
    nmun_t = nc.dram_tensor("nmun", (128, 1), F32, kind="ExternalInput")
    rscr = nc.dram_tensor("rscr", (S, 8, 512), F32, kind="Internal")
    out_t = nc.dram_tensor("res", (COUT, H, W), F32, kind="ExternalOutput")

    with TileContext(nc) as tc:
        with (
            tc.tile_pool(name="consts", bufs=1) as cpool,
            tc.tile_pool(name="xcol", bufs=2) as xcol_pool,
            tc.tile_pool(name="q1p", bufs=2) as q1_pool,
            tc.tile_pool(name="q2", bufs=2) as q2_pool,
            tc.tile_pool(name="q", bufs=2) as q_pool,
            tc.tile_pool(name="e4", bufs=2) as e_pool,
            tc.tile_pool(name="r4", bufs=2) as r_pool,
            tc.tile_pool(name="rbc", bufs=2) as rbc_pool,
            tc.tile_pool(name="resf", bufs=3) as res_pool,
            tc.tile_pool(name="pc1", bufs=2, space="PSUM") as psum_c1,
            tc.tile_pool(name="pc2", bufs=2, space="PSUM") as psum_c2,
            tc.tile_pool(name="pc3", bufs=1, space="PSUM") as psum_c3,
            tc.tile_pool(name="psb", bufs=3, space="PSUM") as psum_sb,
        ):
            w1cT = cpool.tile([27, Q], F16)
            nc.sync.dma_start(w1cT[:, :], w1c_t[:, :])
            waT = cpool.tile([128, 384], F16)
            nc.sync.dma_start(waT[:, :], wa_t[:, :])
            w3mT = cpool.tile([Q, Q], F16)
            nc.sync.dma_start(w3mT[:, :], w3m_t[:, :])
            mu2T = cpool.tile([Q, NC], F16)
            nc.sync.dma_start(mu2T[:, :], mu2_t[:, :])
            lb4T = cpool.tile([128, COUT], BF16)
            nc.sync.dma_start(lb4T[:, :], lb4_t[:, :])
            onesT = cpool.tile([128, 4], BF16)
            nc.sync.dma_start(onesT[:, :], ones_t[:, :])
            b1T = cpool.tile([Q, 1], F32)
            nc.sync.dma_start(b1T[:, :], b1_t[:, :])
            b2T = cpool.tile([Q, 1], F32)
            nc.sync.dma_start(b2T[:, :], b2_t[:, :])
            b3T = cpool.tile([Q, 1], F32)
            nc.sync.dma_start(b3T[:, :], b3_t[:, :])
            nmunT = cpool.tile([128, 1], F32)
            nc.sync.dma_start(nmunT[:, :], nmun_t[:, :])

            for s in range(S):
                r0 = R * s
                if s == 0:
                    A, woff, rb = 0, 1, 0
                elif s == S - 1:
                    A, woff, rb = H - 18, 0, 1
                else:
                    A, woff, rb = r0 - 1, 0, 0

                # ---- conv1: im2col (K=27) + 9 group matmuls ----
                xcol = xcol_pool.tile([27, 18, 256], F16)
                for dy in range(3):
                    for dx in range(3):
                        p = (3 * dy + dx) * 3
                        nc.sync.dma_start(
                            xcol[p : p + 3, :, :],
                            xpad_t[:, A + dy : A + dy + 18, dx : dx + 256],
                        )
                q1p = q1_pool.tile([128, 19, 258], F16)
                for G in range(9):
                    pc1 = psum_c1.tile([64, 512], F32)
                    nc.tensor.matmul(
                        pc1[:, :], w1cT[:, :], xcol[:, 2 * G : 2 * G + 2, :],
                        start=True, stop=True,
                    )
                    a = 2 * G + woff
                    nc.scalar.activation(
                        q1p[0:64, a : a + 2, 1:257],
                        pc1[:, :].rearrange("p (r c) -> p r c", r=2),
                        ACT_F.Relu, bias=b1T[:, :], scale=1.0,
                    )
                # replicate-pad edges
                if s == 0:
                    nc.vector.tensor_copy(q1p[0:64, 0:1, 1:257], q1p[0:64, 1:2, 1:257])
                if s == S - 1:
                    nc.vector.tensor_copy(q1p[0:64, 18:19, 1:257], q1p[0:64, 17:18, 1:257])
                nc.vector.tensor_copy(q1p[0:64, :, 0:1], q1p[0:64, :, 1:2])
                nc.vector.tensor_copy(q1p[0:64, :, 257:258], q1p[0:64, :, 256:257])
                # row-shifted duplicate on partitions 64-127 (for dy=0/1 K-packing)
                nc.sync.dma_start(q1p[64:128, 0:18, :], q1p[0:64, 1:19, :])

                # ---- conv2: 6 matmuls per 2-row group ----
                q2t = q2_pool.tile([64, 4096], F16)
                for g in range(8):
                    pc2 = psum_c2.tile([64, 512], F32)
                    for dx in range(3):
                        nc.tensor.matmul(
                            pc2[:, :], waT[:, 64 * dx : 64 * dx + 64],
                            q1p[:, rb + 2 * g : rb + 2 * g + 2, dx : dx + 256],
                            start=(dx == 0), stop=False,
                        )
                    for dx in range(3):
                        nc.tensor.matmul(
                            pc2[:, :], waT[64:128, 192 + 64 * dx : 256 + 64 * dx],
                            q1p[64:128, rb + 2 * g + 1 : rb + 2 * g + 3, dx : dx + 256],
                            start=False, stop=(dx == 2),
                        )
                    dst = q2t[:, 512 * g : 512 * (g + 1)]
                    if g % 2 == 0:
                        nc.scalar.activation(dst, pc2[:, :], ACT_F.Relu,
                                             bias=b2T[:, :], scale=1.0)
                    else:
                        nc.vector.tensor_scalar(dst, pc2[:, :], b2T[:, :], 0.0,
                                                ALU.add, ALU.max)

                # ---- conv3 (1x1) ----
                qt = q_pool.tile([64, 4096], F16)
                for g in range(8):
                    pc3 = psum_c3.tile([64, 512], F32)
                    nc.tensor.matmul(
                        pc3[:, :], w3mT[:, :], q2t[:, 512 * g : 512 * (g + 1)],
                        start=True, stop=True,
                    )
                    dst = qt[:, 512 * g : 512 * (g + 1)]
                    if g % 2 == 0:
                        nc.vector.tensor_scalar_add(dst, pc3[:, :], b3T[:, :])
                    else:
                        nc.scalar.activation(dst, pc3[:, :], ACT_F.Identity,
                                             bias=b3T[:, :], scale=1.0)

                # ---- stage B: logits, exp, sum, recip, label recombination ----
                for Qd in range(2):
                    ps = psum_sb.tile([128, 512], F32, tag="sb")
                    for j in range(4):
                        g = 4 * Qd + j
                        nc.tensor.matmul(
                            ps[32 * j : 32 * j + 32, :], mu2T[:, :],
                            qt[:, 512 * g : 512 * (g + 1)],
                            start=True, stop=True, tile_position=(0, 32 * j),
                        )
                    e4 = e_pool.tile([128, 512], BF16)
                    nc.scalar.activation(e4[:, :], ps[:, :], ACT_F.Exp,
                                         bias=nmunT[:, :], scale=1.0)
                    pd = psum_sb.tile([128, 512], F32, tag="sb")
                    nc.tensor.matmul(pd[0:4, :], onesT[:, :], e4[:, :],
                                     start=True, stop=True)
                    r4 = r_pool.tile([4, 512], F32)
                    nc.vector.reciprocal(r4[:, :], pd[0:4, :])
                    nc.sync.dma_start(rscr[s, 4 * Qd : 4 * Qd + 4, :], r4[:, :])
                    for pp in range(2):
                        pr = psum_sb.tile([128, 512], F32, tag="sb")
                        for k in range(2):
                            j = 2 * pp + k
                            nc.tensor.matmul(
                                pr[64 * k : 64 * k + 64, :],
                                lb4T[32 * j : 32 * j + 32, :],
                                e4[32 * j : 32 * j + 32, :],
                                start=True, stop=True,
                                tile_position=(32 * j, 64 * k),
                            )
                        rbc = rbc_pool.tile([128, 512], F32)
                        src = bass.AP(
                            tensor=rscr,
                            offset=(s * 8 + 4 * Qd + 2 * pp) * 512,
                            ap=[[512, 2], [0, 64], [1, 512]],
                        )
                        nc.gpsimd.dma_start(rbc[:, :], src)
                        resf = res_pool.tile([128, 512], F32)
                        nc.vector.tensor_tensor(resf[:, :], pr[:, :], rbc[:, :],
                                                op=ALU.mult)
                        g0 = 4 * Qd + 2 * pp
                        row = r0 + 2 * g0
                        nc.sync.dma_start(
                            out_t[:, row : row + 2, :],
                            resf[0:64, :].rearrange("p (r c) -> p r c", r=2),
                        )
                        nc.sync.dma_start(
                            out_t[:, row + 2 : row + 4, :],
                            resf[64:128, :].rearrange("p (r c) -> p r c", r=2),
                        )
    nc.finalize()
    return nc


def _prep_inputs(x, w1, b1, w2, b2, w3, b3, cluster_mu, cluster_label):
    f16 = np.float16
    bf16 = ml_dtypes.bfloat16
    xpad = np.pad(x, ((0, 0), (0, 0), (1, 1), (1, 1)), mode="edge").astype(f16)
    w1c = np.ascontiguousarray(
        w1.transpose(2, 3, 1, 0).reshape(27, Q).astype(f16))
    # wa: [128, 384]; cols 0-191: dy=0 (rows 0-63) / dy=1 (rows 64-127) taps
    #     cols 192-383: dy=2 taps on rows 64-127
    wa = np.zeros((128, 384), f16)
    for dx in range(3):
        wa[0:64, 64 * dx : 64 * dx + 64] = w2[:, :, 0, dx].T
        wa[64:128, 64 * dx : 64 * dx + 64] = w2[:, :, 1, dx].T
        wa[64:128, 192 + 64 * dx : 256 + 64 * dx] = w2[:, :, 2, dx].T
    w3m = np.ascontiguousarray(w3.reshape(Q, Q).T.astype(f16))
    mu = cluster_mu.reshape(NC, Q).astype(np.float32)
    mu2 = np.ascontiguousarray((2.0 * mu).T.astype(f16))
    lb4 = np.tile(np.ascontiguousarray(cluster_label.T), (4, 1)).astype(bf16)
    onesb = np.zeros((128, 4), bf16)
    for j in range(4):
        onesb[32 * j : 32 * j + 32, j] = 1
    mun = np.sum(mu * mu, axis=1)
    nmun = np.tile(-mun, 4).reshape(128, 1).astype(np.float32)
    shared = {
        "w1c": w1c, "wa": wa, "w3m": w3m, "mu2": mu2, "lb4": lb4,
        "onesb": onesb,
        "b1c": b1.reshape(Q, 1).astype(np.float32),
        "b2c": b2.reshape(Q, 1).astype(np.float32),
        "b3c": b3.reshape(Q, 1).astype(np.float32),
        "nmun": nmun,
    }
    return [{"xpad": np.ascontiguousarray(xpad[b]), **shared} for b in range(B)]


def run(inputs, trace=False, **trace_kwargs):
    """Build (cached), run on 8 cores, return (output, BassKernelResults)."""
    if "nc" not in _cache:
        _cache["nc"] = _build()
    in_maps = _prep_inputs(**{k: np.asarray(v) for k, v in inputs.items()})
    res = bass_utils.run_bass_kernel_spmd(
        _cache["nc"], in_maps, core_ids=list(range(B)), trace=trace, **trace_kwargs
    )
    out = np.stack([res.results[b]["res"] for b in range(B)]).astype(np.float32)
    return out, res


def kernel(**inputs):
    out, _ = run(inputs)
    return out
